# revision 10
# baseline (speedup 1.0000x reference)
"""BitLinearV2 Trainium2 kernel.

Computes: out = input @ (ternarize(weight, threshold) * scale[:, None]).T + bias
  input  [4, 2048, 4096] f32
  weight [11008, 4096] f32, threshold [11008, 1], scale [11008], bias [11008]
  out    [4, 2048, 11008] f32

Strategy: column-parallel over 8 NeuronCores (each core owns 1376 output
features).  Input (cast to bf16, transposed to feature-major) is replicated;
weights are ternarized on-device (ScalarE Abs/Sign + VectorE compare/mult)
into a resident bf16 SBUF buffer; the big matmul accumulates over K=4096 in
32 PSUM passes; scale+bias applied to PSUM in f32 on VectorE.
"""

import numpy as np
import ml_dtypes

B, S, I, O = 4, 2048, 4096, 11008
T = B * S               # 8192 tokens
NCORES = 8
OSH = O // NCORES       # 1376 out features per core
KT = I // 128           # 32 contraction slabs
TT = T // 128           # 64 token tiles
O_SLICES = [(0, 512), (512, 512), (1024, 352)]

_CACHE = {}


def _build_nc():
    import concourse.bass as bass
    import concourse.bacc as bacc
    import concourse.mybir as mybir
    import concourse.tile as tile

    nc = bacc.Bacc()
    x_d = nc.dram_tensor("x", [TT, 128, KT, 128], mybir.dt.bfloat16, kind="ExternalInput")
    w_d = nc.dram_tensor("w", [KT, 128, OSH], mybir.dt.float32, kind="ExternalInput")
    thr_d = nc.dram_tensor("thr", [128, OSH], mybir.dt.float32, kind="ExternalInput")
    scale_d = nc.dram_tensor("scale", [128, OSH], mybir.dt.float32, kind="ExternalInput")
    bias_d = nc.dram_tensor("bias", [128, OSH], mybir.dt.float32, kind="ExternalInput")
    out_d = nc.dram_tensor("out", [T, OSH], mybir.dt.float32, kind="ExternalOutput")

    AF = mybir.ActivationFunctionType
    ALU = mybir.AluOpType

    with tile.TileContext(nc) as tc:
        with (
            tc.tile_pool(name="consts", bufs=1) as consts,
            tc.tile_pool(name="wstage", bufs=2) as wstage,
            tc.tile_pool(name="tmps", bufs=2) as tmps,
            tc.tile_pool(name="xin", bufs=3) as xin,
            tc.tile_pool(name="oout", bufs=3) as oout,
            tc.tile_pool(name="psum", bufs=2, space="PSUM") as psum,
        ):
            # per-out-feature vectors, pre-replicated across partitions on host
            thr_b = consts.tile([128, OSH], mybir.dt.float32, tag="thr_b")
            scale_b = consts.tile([128, OSH], mybir.dt.float32, tag="scale_b")
            bias_b = consts.tile([128, OSH], mybir.dt.float32, tag="bias_b")
            nc.sync.dma_start(out=thr_b, in_=thr_d[:])
            nc.sync.dma_start(out=scale_b, in_=scale_d[:])
            nc.sync.dma_start(out=bias_b, in_=bias_d[:])

            # ternarize weight shard into resident bf16 buffer (feature-major)
            w_sb = consts.tile([128, KT, OSH], mybir.dt.bfloat16, tag="w_sb")
            for k in range(KT):
                wf = wstage.tile([128, OSH], mybir.dt.float32, tag="wf")
                nc.sync.dma_start(out=wf, in_=w_d[k])
                aw = tmps.tile([128, OSH], mybir.dt.float32, tag="aw")
                nc.scalar.activation(aw, wf, AF.Abs)
                sg = tmps.tile([128, OSH], mybir.dt.bfloat16, tag="sg")
                nc.scalar.activation(sg, wf, AF.Sign)
                m = tmps.tile([128, OSH], mybir.dt.bfloat16, tag="m")
                nc.vector.tensor_tensor(m, aw, thr_b, ALU.is_ge)
                nc.vector.tensor_tensor(w_sb[:, k, :], m, sg, ALU.mult)

            for t in range(TT):
                xt = xin.tile([128, KT, 128], mybir.dt.bfloat16, tag="xt")
                nc.sync.dma_start(out=xt, in_=x_d[t])
                ot = oout.tile([128, OSH], mybir.dt.float32, tag="ot")
                pts = [
                    psum.tile([128, 512], mybir.dt.float32, tag=f"pt{j}", name=f"pt{j}")
                    for j in range(len(O_SLICES))
                ]
                for k in range(KT):
                    for j, (o0, ow) in enumerate(O_SLICES):
                        nc.tensor.matmul(
                            pts[j][:, :ow],
                            xt[:, k, :],
                            w_sb[:, k, o0 : o0 + ow],
                            start=(k == 0),
                            stop=(k == KT - 1),
                        )
                for j, (o0, ow) in enumerate(O_SLICES):
                    nc.vector.tensor_tensor(
                        ot[:, o0 : o0 + ow], pts[j][:, :ow],
                        scale_b[:, o0 : o0 + ow], ALU.mult,
                    )
                    nc.vector.tensor_tensor(
                        ot[:, o0 : o0 + ow], ot[:, o0 : o0 + ow],
                        bias_b[:, o0 : o0 + ow], ALU.add,
                    )
                nc.sync.dma_start(out=out_d[t * 128 : (t + 1) * 128, :], in_=ot)

    nc.compile()
    return nc


def _get_nc():
    if "nc" not in _CACHE:
        _CACHE["nc"] = _build_nc()
    return _CACHE["nc"]


def _run(inputs, trace=False, tmpdir=None):
    from concourse.bass_utils import run_bass_kernel_spmd

    x = np.asarray(inputs["input"], dtype=np.float32)
    w = np.asarray(inputs["weight"], dtype=np.float32)
    scale = np.asarray(inputs["scale"], dtype=np.float32)
    thr = np.asarray(inputs["threshold"], dtype=np.float32).reshape(O)
    bias = np.asarray(inputs["bias"], dtype=np.float32)

    # [T, I] -> [TT, p=128(feat), KT, tl=128(tok)] bf16, contiguous per partition
    x2 = x.reshape(T, I).astype(ml_dtypes.bfloat16)
    xh = np.ascontiguousarray(x2.reshape(TT, 128, KT, 128).transpose(0, 3, 2, 1))

    in_maps = []
    for c in range(NCORES):
        sl = slice(c * OSH, (c + 1) * OSH)
        wh = np.ascontiguousarray(w[sl].T).reshape(KT, 128, OSH)
        in_maps.append(
            {
                "x": xh,
                "w": wh,
                "thr": np.ascontiguousarray(np.broadcast_to(thr[sl], (128, OSH))),
                "scale": np.ascontiguousarray(np.broadcast_to(scale[sl], (128, OSH))),
                "bias": np.ascontiguousarray(np.broadcast_to(bias[sl], (128, OSH))),
            }
        )

    nc = _get_nc()
    res = run_bass_kernel_spmd(
        nc, in_maps, list(range(NCORES)), trace=trace, tmpdir=tmpdir
    )
    out = np.concatenate([res.results[c]["out"] for c in range(NCORES)], axis=1)
    return out.reshape(B, S, O), res


def kernel(**inputs) -> np.ndarray:
    out, _ = _run(inputs, trace=False)
    return out
